# revision 2
# baseline (speedup 1.0000x reference)
"""Trainium2 Bass kernel for nn_CollectiveFusionAttention (segment_reduce).

Math (reference):
    w[e]   = sigmoid(mean(pos_embed[e]))*(1 + mean(sigmoid(betas[e])))
    num    = einsum('e,edbp,ep->dbp', w, V, m)
    den[p] = max(sum_e w[e]*m[e,p], 1e-6)
    out    = (num/den).sum(d)/5/|T|
           = (sum_e w[e]*m[e,p]*sum_d V[e,d,b,p]) / (5*|T|*max(den_raw,eps))

Sharding: patches (last dim) across 8 cores. Per core (Pc = 8192):
  - partition q = e*8+b (E*B = 128 = SBUF partitions)
  - sum_d V via 5 accumulating SWDGE DMAs (SDMA inline CCE add) -> acc [128, Pc]
  - mask broadcast to [128, Pc] u8 on host, cast u8->f32 during DMA
  - M1 = acc * mask (DVE);  num = Aw^T @ M1, den = Dw^T @ mask (PE, K=128)
    with Aw[q, q%8] = w[q//8], Dw[q, :] = w[q//8]*5|T|/8
  - out = num * reciprocal(max(den, 5|T|*eps))  (DVE)
"""

import os
import sys

import numpy as np

for _p in ("/root/.axon_site/_ro/trn_rl_repo", "/opt/trn_rl_repo"):
    if os.path.isdir(_p) and _p not in sys.path:
        sys.path.append(_p)

import concourse.bacc as bacc
import concourse.mybir as mybir
import concourse.tile as tile
from concourse.bass_utils import run_bass_kernel_spmd

E, D, B, P = 16, 5, 8, 65536
EXPERT_DIM = 128
N_BETAS = 4
EPS = 1e-6
NCORES = 8
PC = P // NCORES          # 8192 patches per core
G = 4                     # column groups per core
GC = PC // G              # 2048 columns per group
MMN = 512                 # matmul moving free dim (fp32 max)
NK = GC // MMN            # chunks per group

F32 = mybir.dt.float32
U8 = mybir.dt.uint8
ALU = mybir.AluOpType
ACTF = mybir.ActivationFunctionType

_cache = {}


def _build():
    nc = bacc.Bacc("TRN2", target_bir_lowering=False, debug=False,
                   num_devices=NCORES)

    v_dram = nc.dram_tensor("v", [E, D, B, PC], F32, kind="ExternalInput")
    m_dram = nc.dram_tensor("m", [128, PC], U8, kind="ExternalInput")
    pe_dram = nc.dram_tensor("pe", [E, EXPERT_DIM], F32, kind="ExternalInput")
    be_dram = nc.dram_tensor("be", [E, N_BETAS], F32, kind="ExternalInput")
    t_dram = nc.dram_tensor("t", [1], F32, kind="ExternalInput")
    out_dram = nc.dram_tensor("out", [B, PC], F32, kind="ExternalOutput")

    # constant matrices
    sel = np.zeros((128, 8), dtype=np.float32)
    sel[np.arange(128), np.arange(128) % 8] = 1.0
    mb = np.zeros((E, 128), dtype=np.float32)
    mb[np.arange(128) // 8, np.arange(128)] = 1.0
    sel_dram = nc.inline_tensor(sel, name="c_sel")
    eighth_dram = nc.inline_tensor(np.full((128, 8), 0.125, np.float32),
                                   name="c_eighth")
    mb_dram = nc.inline_tensor(mb, name="c_mb")
    o18_dram = nc.inline_tensor(np.ones((1, 8), np.float32), name="c_o18")
    o1128_dram = nc.inline_tensor(np.ones((1, 128), np.float32), name="c_o1128")

    with tile.TileContext(nc) as tc:
        with (
            tc.tile_pool(name="sb", bufs=1) as pool,
            tc.tile_pool(name="ps", bufs=1, space="PSUM") as psum,
            tc.tile_pool(name="ps_mm", bufs=2, space="PSUM") as psmm,
        ):
            # ---- one-time setup: per-expert weights ----
            sel_t = pool.tile([128, 8], F32, tag="sel")
            eighth_t = pool.tile([128, 8], F32, tag="eighth")
            mb_t = pool.tile([E, 128], F32, tag="mb")
            o18_t = pool.tile([1, 8], F32, tag="o18")
            o1128_t = pool.tile([1, 128], F32, tag="o1128")
            nc.sync.dma_start(sel_t[:], sel_dram[:])
            nc.sync.dma_start(eighth_t[:], eighth_dram[:])
            nc.sync.dma_start(mb_t[:], mb_dram[:])
            nc.sync.dma_start(o18_t[:], o18_dram[:])
            nc.sync.dma_start(o1128_t[:], o1128_dram[:])

            pe_t = pool.tile([E, EXPERT_DIM], F32, tag="pe")
            be_t = pool.tile([E, N_BETAS], F32, tag="be")
            temp_t = pool.tile([1, 1], F32, tag="temp")
            nc.sync.dma_start(pe_t[:], pe_dram[:])
            nc.sync.dma_start(be_t[:], be_dram[:])
            nc.sync.dma_start(temp_t[:], t_dram[:])

            psum_s = pool.tile([E, 1], F32, tag="psum_s")
            pw = pool.tile([E, 1], F32, tag="pw")
            bs = pool.tile([E, N_BETAS], F32, tag="bs")
            bsum = pool.tile([E, 1], F32, tag="bsum")
            bw = pool.tile([E, 1], F32, tag="bw")
            w16 = pool.tile([E, 1], F32, tag="w16")
            nc.vector.reduce_sum(out=psum_s[:], in_=pe_t[:],
                                 axis=mybir.AxisListType.X)
            nc.scalar.activation(pw[:], psum_s[:], ACTF.Sigmoid,
                                 scale=1.0 / EXPERT_DIM)
            nc.scalar.activation(bs[:], be_t[:], ACTF.Sigmoid)
            nc.vector.reduce_sum(out=bsum[:], in_=bs[:],
                                 axis=mybir.AxisListType.X)
            # 1 + mean(sigmoid(betas))
            nc.scalar.activation(bw[:], bsum[:], ACTF.Copy, bias=1.0,
                                 scale=1.0 / N_BETAS)
            nc.vector.tensor_tensor(out=w16[:], in0=pw[:], in1=bw[:],
                                    op=ALU.mult)

            # t5 = 5*|T|, t5e = 5*eps*|T|
            t5 = pool.tile([1, 1], F32, tag="t5")
            t5e = pool.tile([1, 1], F32, tag="t5e")
            nc.scalar.activation(t5[:], temp_t[:], ACTF.Abs, scale=5.0)
            nc.scalar.activation(t5e[:], temp_t[:], ACTF.Abs, scale=5.0 * EPS)

            # broadcasts via tiny matmuls
            w128_ps = psum.tile([128, 1], F32, tag="w128ps")
            t5128_ps = psum.tile([128, 1], F32, tag="t5128ps")
            sc8_ps = psum.tile([8, 1], F32, tag="sc8ps")
            nc.tensor.matmul(w128_ps[:], mb_t[:], w16[:], start=True, stop=True)
            nc.tensor.matmul(t5128_ps[:], o1128_t[:], t5[:], start=True,
                             stop=True)
            nc.tensor.matmul(sc8_ps[:], o18_t[:], t5e[:], start=True, stop=True)
            w128 = pool.tile([128, 1], F32, tag="w128")
            t5128 = pool.tile([128, 1], F32, tag="t5128")
            sc8 = pool.tile([8, 1], F32, tag="sc8")
            nc.vector.tensor_copy(w128[:], w128_ps[:])
            nc.vector.tensor_copy(t5128[:], t5128_ps[:])
            nc.vector.tensor_copy(sc8[:], sc8_ps[:])

            aw_t = pool.tile([128, 8], F32, tag="aw")
            dw_t = pool.tile([128, 8], F32, tag="dw")
            nc.vector.tensor_scalar_mul(aw_t[:], sel_t[:], w128[:])
            nc.vector.tensor_scalar(out=dw_t[:], in0=eighth_t[:],
                                    scalar1=w128[:], scalar2=t5128[:],
                                    op0=ALU.mult, op1=ALU.mult)

            # ---- main data path ----
            accs, maskts, outts = [], [], []
            for g in range(G):
                accs.append(pool.tile([128, GC], F32, name=f"acc{g}", tag=f"acc{g}"))
                maskts.append(pool.tile([128, GC], F32, name=f"mask{g}", tag=f"mask{g}"))
                outts.append(pool.tile([B, GC], F32, name=f"out{g}", tag=f"out{g}"))

            # mask cast-DMAs (u8 -> f32), independent
            for g in range(G):
                nc.gpsimd.dma_start(maskts[g][:],
                                    m_dram[:, g * GC:(g + 1) * GC])
            # V accumulate chains, round-major so chains interleave on SDMA
            for d in range(D):
                for g in range(G):
                    nc.gpsimd.dma_start(
                        accs[g][:],
                        v_dram[:, d, :, g * GC:(g + 1) * GC],
                        accum_op=(ALU.bypass if d == 0 else ALU.add),
                    )

            for g in range(G):
                acc, maskt, outt = accs[g], maskts[g], outts[g]
                nc.vector.tensor_tensor(out=acc[:], in0=acc[:], in1=maskt[:],
                                        op=ALU.mult)
                for k in range(NK):
                    cs = slice(k * MMN, (k + 1) * MMN)
                    den_ps = psmm.tile([8, MMN], F32, tag="den")
                    num_ps = psmm.tile([8, MMN], F32, tag="num")
                    nc.tensor.matmul(den_ps[:], dw_t[:], maskt[:, cs],
                                     start=True, stop=True)
                    nc.tensor.matmul(num_ps[:], aw_t[:], acc[:, cs],
                                     start=True, stop=True)
                    dclamp = pool.tile([8, MMN], F32, tag="dclamp")
                    rec = pool.tile([8, MMN], F32, tag="rec")
                    nc.vector.tensor_scalar(out=dclamp[:], in0=den_ps[:],
                                            scalar1=sc8[:], scalar2=None,
                                            op0=ALU.max)
                    nc.vector.reciprocal(rec[:], dclamp[:])
                    nc.vector.tensor_tensor(out=outt[:, cs], in0=num_ps[:],
                                            in1=rec[:], op=ALU.mult)
                nc.sync.dma_start(out_dram[:, g * GC:(g + 1) * GC], outt[:])

    nc.compile()
    return nc


def kernel(V, masks, betas, expert_pos_embed, temperature, num_patches):
    assert int(num_patches) == P and V.shape == (E, D, B, P)
    if "nc" not in _cache:
        _cache["nc"] = _build()
    nc = _cache["nc"]

    V = np.ascontiguousarray(V, dtype=np.float32)
    masks_u8 = np.ascontiguousarray(masks).astype(np.uint8)
    in_maps = []
    for i in range(NCORES):
        sl = slice(i * PC, (i + 1) * PC)
        in_maps.append({
            "v": np.ascontiguousarray(V[:, :, :, sl]),
            "m": np.ascontiguousarray(np.repeat(masks_u8[:, sl], B, axis=0)),
            "pe": np.ascontiguousarray(expert_pos_embed, dtype=np.float32),
            "be": np.ascontiguousarray(betas, dtype=np.float32),
            "t": np.ascontiguousarray(temperature, dtype=np.float32),
        })
    res = run_bass_kernel_spmd(nc, in_maps, core_ids=list(range(NCORES)))
    return np.concatenate([res.results[i]["out"] for i in range(NCORES)],
                          axis=1)
